# revision 34
# baseline (speedup 1.0000x reference)
"""BertEmbeddings (7-way embedding sum + Time2Vec + LayerNorm) on 8 TRN2 cores.

Data-parallel: core i handles batch row i (2048 tokens); tables replicated.
Token t lives at SBUF [partition p = t % 128, column w = t // 128]; chunks of
C=4 columns (512 tokens). Per chunk:
  - word/npi/posi rows arrive via dma_gather (int16 indices, host-packed into
    the [16-partition wrap] layout the SWDGE expects)
  - modal+seg come from a one-hot matmul on TensorE against a combined 32-row
    bf16 table (one-hot built directly in [vocab-partition, token-free]
    orientation - no transpose needed in this layout)
  - Time2Vec: one fused scalar_tensor_tensor (w'*tau+b') per column with
    w' = w/2pi, then d = x - round(x) (pool-engine convert) and one ACT pass
    sin(2pi*d) (HW sin LUT is only valid on [-pi, pi])
  - LayerNorm via bn_stats/bn_aggr + fused (x-mean)*rstd tensor_scalar

Self-contained: hardcodes shapes; kernel(**inputs) takes full unsharded
inputs, returns the full [8, 2048, 768] float32 output.
"""

import math

import numpy as np

B, S, H = 8, 2048, 768
VOCAB, MODAL_V, SEG_V, NPI_V, MAX_POS = 32000, 16, 4, 10000, 2048
P = 128
COLS = S // P        # 16 token-columns
NCHUNK = 4
C = COLS // NCHUNK   # 4 columns per chunk
NIDX = P * C         # 512 tokens per chunk
LN_EPS = 1e-12
INV_2PI = 1.0 / (2.0 * math.pi)
TWO_PI = 2.0 * math.pi
KMS = 64             # modal rows 0..15, seg rows 32..35 (base-32 aligned), zero pad

_cache = {}


def _build(use_gamma_beta: bool):
    import concourse.bacc as bacc
    import concourse.bass as bass
    import concourse.tile as tile
    from concourse import mybir
    from concourse.masks import make_identity
    from contextlib import ExitStack

    f32 = mybir.dt.float32
    bf16 = mybir.dt.bfloat16
    i32 = mybir.dt.int32
    i16 = mybir.dt.int16
    Alu = mybir.AluOpType
    Act = mybir.ActivationFunctionType

    nc = bacc.Bacc("TRN2", target_bir_lowering=False, debug=False,
                   dynamic_dma_scratch_size=24576, num_swdge_queues=2)

    d_widx = nc.dram_tensor("word_idx16", [P, NCHUNK, 32], i16, kind="ExternalInput")
    d_nidx = nc.dram_tensor("npi_idx16", [P, NCHUNK, 32], i16, kind="ExternalInput")
    d_pidx = nc.dram_tensor("posi_idx16", [P, NCHUNK, 32], i16, kind="ExternalInput")
    d_modal_ids = nc.dram_tensor("modalities_ids", [S], i32, kind="ExternalInput")
    d_seg_ids = nc.dram_tensor("seg_ids", [S], i32, kind="ExternalInput")
    d_age_tau = nc.dram_tensor("age_tau", [S], f32, kind="ExternalInput")
    d_delay_tau = nc.dram_tensor("delays_tau", [S], f32, kind="ExternalInput")
    d_word_tbl = nc.dram_tensor("word_table", [VOCAB, H], bf16, kind="ExternalInput")
    d_modal_tbl = nc.dram_tensor("modalities_table", [MODAL_V, H], bf16, kind="ExternalInput")
    d_seg_tbl = nc.dram_tensor("seg_table", [SEG_V, H], bf16, kind="ExternalInput")
    d_npi_tbl = nc.dram_tensor("npi_table", [NPI_V, H], bf16, kind="ExternalInput")
    d_posi_tbl = nc.dram_tensor("posi_table", [MAX_POS, H], bf16, kind="ExternalInput")
    d_age_w = nc.dram_tensor("age_w", [1, H - 1], f32, kind="ExternalInput")
    d_age_b = nc.dram_tensor("age_b", [H - 1], f32, kind="ExternalInput")
    d_age_w0 = nc.dram_tensor("age_w0", [1, 1], f32, kind="ExternalInput")
    d_age_b0 = nc.dram_tensor("age_b0", [1], f32, kind="ExternalInput")
    d_del_w = nc.dram_tensor("delay_w", [1, H - 1], f32, kind="ExternalInput")
    d_del_b = nc.dram_tensor("delay_b", [H - 1], f32, kind="ExternalInput")
    d_del_w0 = nc.dram_tensor("delay_w0", [1, 1], f32, kind="ExternalInput")
    d_del_b0 = nc.dram_tensor("delay_b0", [1], f32, kind="ExternalInput")
    if use_gamma_beta:
        d_gamma = nc.dram_tensor("ln_gamma", [H], f32, kind="ExternalInput")
        d_beta = nc.dram_tensor("ln_beta", [H], f32, kind="ExternalInput")
    d_out = nc.dram_tensor("out", [S, H], f32, kind="ExternalOutput")

    def bcast_rows(handle, n, count, offset=0):
        ap = handle.ap()
        return bass.AP(tensor=ap.tensor, offset=offset, ap=[[0, n], [1, count]])

    with tile.TileContext(nc) as tc, ExitStack() as ctx:
        singles = ctx.enter_context(tc.tile_pool(name="singles", bufs=1))
        work = ctx.enter_context(tc.tile_pool(name="work", bufs=2))
        work3 = ctx.enter_context(tc.tile_pool(name="work3", bufs=3))
        gwork = ctx.enter_context(tc.tile_pool(name="gwork", bufs=3))
        psum = ctx.enter_context(tc.tile_pool(name="psum", bufs=4, space="PSUM"))

        # ---- per-core constants (tau path first: first chunk's t2v needs it) ----
        identity16 = singles.tile([16, 16], f32)
        make_identity(nc, identity16[:])

        # taus arrive [16, 128] (contiguous), transposed on PE into [128, 16]
        # columns 0..15 = age, 16..31 = delay.
        tau_raw = singles.tile([16, 2, P], f32)
        nc.sync.dma_start(
            out=tau_raw[:, 0, :], in_=d_age_tau.ap().rearrange("(w p) -> w p", p=P)
        )
        nc.sync.dma_start(
            out=tau_raw[:, 1, :], in_=d_delay_tau.ap().rearrange("(w p) -> w p", p=P)
        )
        tau_ps = psum.tile([P, 2 * COLS], f32, tag="mm")
        nc.tensor.transpose(
            out=tau_ps[:, 0:COLS], in_=tau_raw[:, 0, :], identity=identity16[:]
        )
        nc.tensor.transpose(
            out=tau_ps[:, COLS : 2 * COLS], in_=tau_raw[:, 1, :], identity=identity16[:]
        )
        taus = singles.tile([P, 2 * COLS], f32)
        nc.vector.tensor_copy(out=taus[:], in_=tau_ps[:])

        wi16 = singles.tile([P, NCHUNK, 32], i16)
        ni16 = singles.tile([P, NCHUNK, 32], i16)
        pi16 = singles.tile([P, NCHUNK, 32], i16)
        nc.sync.dma_start(out=wi16[:], in_=d_widx.ap())
        nc.sync.dma_start(out=ni16[:], in_=d_nidx.ap())
        nc.sync.dma_start(out=pi16[:], in_=d_pidx.ap())

        identity_bf = singles.tile([P, P], bf16)
        make_identity(nc, identity_bf[:])

        # one-hot for all 16 columns; pad rows zeroed once and never rewritten
        onehot = singles.tile([KMS, COLS, P], bf16)
        nc.vector.memset(onehot[:, :, :], 0.0)

        # modal/seg ids for all chunks, broadcast into the one-hot orientation
        ids_all = singles.tile([KMS, COLS, P], i32)
        nc.sync.dma_start(
            out=ids_all[0:MODAL_V, :, :],
            in_=bass.AP(
                tensor=d_modal_ids.ap().tensor, offset=0,
                ap=[[0, MODAL_V], [P, COLS], [1, P]],
            ),
        )
        nc.sync.dma_start(
            out=ids_all[32 : 32 + SEG_V, :, :],
            in_=bass.AP(
                tensor=d_seg_ids.ap().tensor, offset=0,
                ap=[[0, SEG_V], [P, COLS], [1, P]],
            ),
        )

        # Time2Vec params broadcast across partitions; cols 0..766 hold w/2pi
        # and b/2pi (range reduction), col 767 the raw w0/b0 linear feature.
        w_age = singles.tile([P, H], f32)
        b_age = singles.tile([P, H], f32)
        w_del = singles.tile([P, H], f32)
        b_del = singles.tile([P, H], f32)
        for dw, db, dw0, db0, wt, bt in (
            (d_age_w, d_age_b, d_age_w0, d_age_b0, w_age, b_age),
            (d_del_w, d_del_b, d_del_w0, d_del_b0, w_del, b_del),
        ):
            nc.sync.dma_start(out=wt[:, 0 : H - 1], in_=bcast_rows(dw, P, H - 1))
            nc.sync.dma_start(out=wt[:, H - 1 : H], in_=bcast_rows(dw0, P, 1))
            nc.sync.dma_start(out=bt[:, 0 : H - 1], in_=bcast_rows(db, P, H - 1))
            nc.sync.dma_start(out=bt[:, H - 1 : H], in_=bcast_rows(db0, P, 1))
            nc.scalar.activation(
                out=wt[:, 0 : H - 1], in_=wt[:, 0 : H - 1], func=Act.Copy,
                bias=0.0, scale=INV_2PI,
            )
            nc.scalar.activation(
                out=bt[:, 0 : H - 1], in_=bt[:, 0 : H - 1], func=Act.Copy,
                bias=0.0, scale=INV_2PI,
            )

        # combined modal+seg table: rows 0..15 modal, 32..35 seg, rest zero
        ctbl = singles.tile([KMS, H], bf16)
        nc.vector.memset(ctbl[:], 0.0)
        nc.sync.dma_start(out=ctbl[0:MODAL_V, :], in_=d_modal_tbl.ap())
        nc.sync.dma_start(out=ctbl[32 : 32 + SEG_V, :], in_=d_seg_tbl.ap())

        # iota over partitions: value p (for modal rows) and p-32 (seg rows)
        iota_p = singles.tile([P, 1], i16)
        nc.gpsimd.iota(iota_p[:], pattern=[[1, 1]], base=0, channel_multiplier=1)
        iota_pm32 = singles.tile([P, 1], i16)
        nc.gpsimd.iota(iota_pm32[:], pattern=[[1, 1]], base=-32, channel_multiplier=1)
        eps_t = singles.tile([P, 1], f32)
        nc.vector.memset(eps_t[:], LN_EPS)
        # b0_age + b0_del for the linear col-767 feature
        b0sum = singles.tile([P, 1], f32)
        nc.vector.tensor_tensor(
            out=b0sum[:], in0=b_age[:, H - 1 : H], in1=b_del[:, H - 1 : H], op=Alu.add
        )

        if use_gamma_beta:
            gamma_t = singles.tile([P, H], f32)
            beta_t = singles.tile([P, H], f32)
            nc.sync.dma_start(out=gamma_t[:], in_=bcast_rows(d_gamma, P, H))
            nc.sync.dma_start(out=beta_t[:], in_=bcast_rows(d_beta, P, H))

        def pbcast(ap1, reps_c, reps_f):
            # [p, 1] tile AP -> [p, reps_c, reps_f] step-0 broadcast
            return bass.AP(
                tensor=ap1.tensor, offset=ap1.offset,
                ap=[ap1.ap[0], [0, reps_c], [0, reps_f]],
            )

        out_v = d_out.ap().rearrange("(k c p) h -> p k c h", p=P, c=C)

        for k in range(NCHUNK):
            w0c = k * C

            # ---- gathers (bf16 rows) ----
            word_bf = gwork.tile([P, C, H], bf16)
            npi_bf = gwork.tile([P, C, H], bf16)
            posi_bf = gwork.tile([P, C, H], bf16)
            for gi, (tbl, idxs, dst) in enumerate((
                (d_word_tbl, wi16, word_bf),
                (d_npi_tbl, ni16, npi_bf),
                (d_posi_tbl, pi16, posi_bf),
            )):
                nc.gpsimd.dma_gather(
                    out_ap=dst[:, :, :], in_ap=tbl.ap(), idxs_ap=idxs[:, k, :],
                    num_idxs=NIDX, num_idxs_reg=NIDX, elem_size=H,
                    queue_num=(k * 3 + gi) % 2,
                )

            # ---- Time2Vec (age / delay): sin cols into bf16, col 767 zeroed
            # (the f32 linear feature is added separately) ----
            age_bf = work3.tile([P, C, H], bf16)
            del_bf = work3.tile([P, C, H], bf16)
            for ti, (wt, bt, sbf) in enumerate(
                ((w_age, b_age, age_bf), (w_del, b_del, del_bf))
            ):
                tmp = work.tile([P, C, H - 1], f32, tag="t2v_tmp")
                for c in range(C):
                    j = ti * COLS + w0c + c
                    nc.vector.scalar_tensor_tensor(
                        out=tmp[:, c, :], in0=wt[:, 0 : H - 1],
                        scalar=taus[:, j : j + 1],
                        in1=bt[:, 0 : H - 1], op0=Alu.mult, op1=Alu.add,
                    )
                kint = work.tile([P, C, H - 1], i16, tag="kint")
                nc.vector.tensor_copy(out=kint[:, :, :], in_=tmp[:, :, :])
                nc.vector.tensor_tensor(
                    out=tmp[:, :, :], in0=tmp[:, :, :], in1=kint[:, :, :],
                    op=Alu.subtract,
                )
                nc.scalar.activation(
                    out=sbf[:, :, 0 : H - 1], in_=tmp[:, :, :],
                    func=Act.Sin, scale=TWO_PI,
                )
                nc.vector.memset(sbf[:, :, H - 1 : H], 0.0)

            # v2 = tau_a*w0_a + b0_a + tau_d*w0_d + b0_d for col 767 (f32)
            v2t = work.tile([P, C], f32)
            nc.vector.tensor_scalar(
                out=v2t[:], in0=taus[:, w0c : w0c + C],
                scalar1=w_age[:, H - 1 : H], scalar2=b0sum[:],
                op0=Alu.mult, op1=Alu.add,
            )
            nc.vector.scalar_tensor_tensor(
                out=v2t[:], in0=taus[:, COLS + w0c : COLS + w0c + C],
                scalar=w_del[:, H - 1 : H], in1=v2t[:],
                op0=Alu.mult, op1=Alu.add,
            )

            # ---- modal+seg one-hot (already in [vocab, token] orientation) ----
            nc.vector.tensor_tensor(
                out=onehot[0:MODAL_V, w0c : w0c + C, :],
                in0=ids_all[0:MODAL_V, w0c : w0c + C, :],
                in1=pbcast(iota_p[0:MODAL_V], C, P), op=Alu.is_equal,
            )
            nc.vector.tensor_tensor(
                out=onehot[32 : 32 + SEG_V, w0c : w0c + C, :],
                in0=ids_all[32 : 32 + SEG_V, w0c : w0c + C, :],
                in1=pbcast(iota_pm32[32 : 32 + SEG_V], C, P),
                op=Alu.is_equal,
            )

            # ---- PE sums modal/seg + age + del + npi + posi into PSUM ----
            emb = work3.tile([P, C, H], f32)
            s0 = work.tile([P, C], f32)
            for c in range(C):
                ms_ps = psum.tile([P, H], f32, tag="mm")
                for n0 in range(0, H, 512):
                    n1 = min(n0 + 512, H)
                    nc.tensor.matmul(
                        out=ms_ps[:, n0:n1], lhsT=onehot[:, w0c + c, :],
                        rhs=ctbl[:, n0:n1], start=True, stop=False,
                    )
                    for rhs_t, last in (
                        (age_bf, False), (del_bf, False), (npi_bf, False),
                        (posi_bf, False), (word_bf, True),
                    ):
                        nc.tensor.matmul(
                            out=ms_ps[:, n0:n1], lhsT=identity_bf[:],
                            rhs=rhs_t[:, c, n0:n1], start=False, stop=last,
                        )
                # emb_c = psum copy (ACT); accum_out gives sum(emb_c) free
                nc.scalar.activation(
                    out=emb[:, c, :], in_=ms_ps[:, :], func=Act.Identity,
                    accum_out=s0[:, c : c + 1],
                )
            # add the f32 linear Time2Vec feature at col 767 (and fix sums)
            nc.vector.tensor_tensor(
                out=emb[:, :, H - 1],
                in0=emb[:, :, H - 1], in1=v2t[:], op=Alu.add,
            )
            nc.vector.tensor_tensor(out=s0[:], in0=s0[:], in1=v2t[:], op=Alu.add)

            # ---- LayerNorm over H per token ----
            # sumsq via ACT Square accumulate; mean/var/rstd in tiny DVE ops
            s2 = work.tile([P, C], f32)
            sq_scratch = work.tile([P, H], f32, tag="sqs")
            for c in range(C):
                nc.scalar.activation(
                    out=sq_scratch[:, :], in_=emb[:, c, :], func=Act.Square,
                    accum_out=s2[:, c : c + 1],
                )
            mean = work.tile([P, C], f32)
            nc.vector.tensor_scalar_mul(out=mean[:], in0=s0[:], scalar1=1.0 / H)
            msq = work.tile([P, C], f32)
            nc.vector.tensor_tensor(out=msq[:], in0=mean[:], in1=mean[:], op=Alu.mult)
            var = work.tile([P, C], f32)
            nc.vector.scalar_tensor_tensor(
                out=var[:], in0=s2[:], scalar=1.0 / H, in1=msq[:],
                op0=Alu.mult, op1=Alu.subtract,
            )
            rstd = work.tile([P, C], f32)
            nc.scalar.activation(
                out=rstd[:], in_=var[:], func=Act.Sqrt, bias=eps_t[:], scale=1.0,
            )
            nc.vector.reciprocal(out=rstd[:], in_=rstd[:])
            negmur = work.tile([P, C], f32)
            nc.vector.scalar_tensor_tensor(
                out=negmur[:], in0=mean[:], scalar=-1.0, in1=rstd[:],
                op0=Alu.mult, op1=Alu.mult,
            )
            for c in range(C):
                nc.scalar.activation(
                    out=emb[:, c, :], in_=emb[:, c, :], func=Act.Identity,
                    scale=rstd[:, c : c + 1], bias=negmur[:, c : c + 1],
                )
            if use_gamma_beta:
                def mid_bcast(ap2, reps):
                    return bass.AP(
                        tensor=ap2.tensor, offset=ap2.offset,
                        ap=[ap2.ap[0], [0, reps], ap2.ap[1]],
                    )
                nc.vector.tensor_tensor(
                    out=emb[:, :, :], in0=emb[:, :, :],
                    in1=mid_bcast(gamma_t[:], C), op=Alu.mult,
                )
                nc.vector.tensor_tensor(
                    out=emb[:, :, :], in0=emb[:, :, :],
                    in1=mid_bcast(beta_t[:], C), op=Alu.add,
                )

            nc.sync.dma_start(out=out_v[:, k, :, :], in_=emb[:, :, :])

    nc.compile()
    return nc


def _get_nc(use_gamma_beta: bool):
    key = ("nc", use_gamma_beta)
    if key not in _cache:
        _cache[key] = _build(use_gamma_beta)
    return _cache[key]


def _f32a(x):
    return np.ascontiguousarray(np.asarray(x), dtype=np.float32)


def _i32a(x):
    return np.ascontiguousarray(np.asarray(x), dtype=np.int32)


def _pack_idx16(ids_row):
    # ids_row [S] -> [P, NCHUNK, 32] int16 in the dma_gather wrap layout:
    # token i of chunk k sits at [i % 16, k, i // 16].
    arr = np.zeros((P, NCHUNK, 32), dtype=np.int16)
    for k in range(NCHUNK):
        blk = ids_row[k * NIDX : (k + 1) * NIDX].reshape(32, 16)
        arr[0:16, k, :] = blk.T.astype(np.int16)
    # each of the 8 GpSimd cores reads the wrap from its own 16-partition group
    return np.tile(arr[0:16], (8, 1, 1))


_last_use_gb = False


def _make_in_maps(inputs, use_gb):
    word_ids = _i32a(inputs["word_ids"]).reshape(B, S)
    modal_ids = _i32a(inputs["modalities_ids"]).reshape(B, S)
    seg_ids = _i32a(inputs["seg_ids"]).reshape(B, S)
    npi_ids = _i32a(inputs["npi_ids"]).reshape(B, S)
    posi_ids = _i32a(inputs["posi_ids"]).reshape(B, S)
    age_tau = _f32a(inputs["age_tau"]).reshape(B, S)
    delay_tau = _f32a(inputs["delays_tau"]).reshape(B, S)

    import ml_dtypes
    bf16np = ml_dtypes.bfloat16

    shared = {
        "word_table": np.ascontiguousarray(
            np.asarray(inputs["word_table"], dtype=np.float32).reshape(VOCAB, H).astype(bf16np)
        ),
        "modalities_table": np.ascontiguousarray(
            np.asarray(inputs["modalities_table"], dtype=np.float32).reshape(MODAL_V, H).astype(bf16np)
        ),
        "seg_table": np.ascontiguousarray(
            np.asarray(inputs["seg_table"], dtype=np.float32).reshape(SEG_V, H).astype(bf16np)
        ),
        "npi_table": np.ascontiguousarray(
            np.asarray(inputs["npi_table"], dtype=np.float32).reshape(NPI_V, H).astype(bf16np)
        ),
        "posi_table": np.ascontiguousarray(
            np.asarray(inputs["posi_table"], dtype=np.float32).reshape(MAX_POS, H).astype(bf16np)
        ),
        "age_w": _f32a(inputs["age_w"]).reshape(1, H - 1),
        "age_b": _f32a(inputs["age_b"]).reshape(H - 1),
        "age_w0": _f32a(inputs["age_w0"]).reshape(1, 1),
        "age_b0": _f32a(inputs["age_b0"]).reshape(1),
        "delay_w": _f32a(inputs["delay_w"]).reshape(1, H - 1),
        "delay_b": _f32a(inputs["delay_b"]).reshape(H - 1),
        "delay_w0": _f32a(inputs["delay_w0"]).reshape(1, 1),
        "delay_b0": _f32a(inputs["delay_b0"]).reshape(1),
    }
    if use_gb:
        shared["ln_gamma"] = _f32a(inputs["ln_gamma"]).reshape(H)
        shared["ln_beta"] = _f32a(inputs["ln_beta"]).reshape(H)

    in_maps = []
    for i in range(B):
        m = dict(shared)
        m.update(
            word_idx16=_pack_idx16(word_ids[i]),
            npi_idx16=_pack_idx16(npi_ids[i]),
            posi_idx16=_pack_idx16(posi_ids[i]),
            modalities_ids=modal_ids[i],
            seg_ids=seg_ids[i],
            age_tau=age_tau[i],
            delays_tau=delay_tau[i],
        )
        in_maps.append(m)
    return in_maps


def kernel(**inputs) -> np.ndarray:
    global _last_use_gb
    from concourse.bass_utils import run_bass_kernel_spmd

    gamma = _f32a(inputs["ln_gamma"])
    beta = _f32a(inputs["ln_beta"])
    use_gb = not (np.all(gamma == 1.0) and np.all(beta == 0.0))
    _last_use_gb = use_gb
    nc = _get_nc(use_gb)
    in_maps = _make_in_maps(inputs, use_gb)
    core_ids = list(range(B))
    res = run_bass_kernel_spmd(nc, in_maps, core_ids)
    out = np.stack([res.results[i]["out"] for i in core_ids], axis=0)
    # device layout is t = w*128 + p per row of [S, H]; out rows are already in
    # token order because the store APs scatter back to token order.
    return out
